# revision 47
# baseline (speedup 1.0000x reference)
"""Trainium2 Bass kernel v3 for masked additive-attention pooling.

Reference math (per batch b):
    whhn = encoding @ W_h.T                            # [B, D]
    M    = tanh(X @ W_y.T + whhn[:, None, :])          # [B, T, D]
    a    = sigmoid(M @ w_a)                            # [B, T]
    e    = exp(a); den = sum(e * mask); w = e * mask / den
    out  = sum_t w[t] * X[t]                           # [B, D]

Sharding: data-parallel over batch B=32 across 8 cores (4 batches/core).
Weights replicated. Host does layout transforms only.

v4: VARIABLE-LENGTH SKIP (-17us, 173.8 -> ~157us mean). Fully-masked
512-token tiles have provably zero attention weight (w = e*mask/den),
so they are never computed: the host sorts batches by valid length and
assigns them to core slots with a shared compile-time shape
cnt=(4,4,4,2) -- every core gets three long batches plus one with <=2
valid tiles (seed-0 has nine such), so all 8 cores run the same
14-tile NEFF instead of 16 tiles. Partially-masked tiles are still
computed (mask zeroes them). Falls back to the dense (4,4,4,4) shape,
compiled on demand, whenever fewer than 8 batches fit in 2 tiles.
Outputs are un-permuted on the host after the gather.

v3 changes vs v2 (183us -> ~173us measured, one-shot NTFF timing has
~+-3us run-to-run jitter; throttled-device outliers reach 200us+):
  - apre (logits): 4-way col-tiled (tile_position) N=256 matmuls, one
    [128,32] wa-chunk stationary per strip (col 0 = w_a chunk, rest 0),
    accumulating 8 e-chunks into 4 PSUM row-strips. The 4 strips run
    truly concurrently (Dstart ~3ns); kills the v2 DR apre stream AND
    the t_cols K=1 transpose matmuls. Strips use memset + start=False
    (never start=True: the bank-wide has_written clear breaks
    interleaved accumulation; overwrite-on-first-touch vs
    accumulate-onto-0 are both correct).
  - strip rows go through ONE DVE 32x32 block-transpose per half
    ([128,256] PSUM -> SBUF, ~450ns); a host-side token permutation
    within each 512-token j-tile (c<->k index swap) makes the
    transposed layout land exactly on the pooling stationary columns.
    x-natural and mask are permuted identically on host; the xt /
    z-stream token order is unchanged.
  - apre for h is emitted as one batched run during h+1's stream
    (pipelined drains); the last half trails per-eb so the tail stays
    short. th stored as plain [128, eb, jl, 512] fp8.
  - wh shipped as fp8 x32 with enc pre-scaled 1/32 on host (halves the
    h0-critical whhn weight traffic); mask shipped bf16; all small
    consts packed into ONE [128,354] bf16 DMA (per-DMA issue cost is
    ~0.6us on an engine queue, so count matters more than bytes).
  - DMA plan: sync = xt0/xt1 quarters then steady x-natural; scalar =
    wy, xt2/3, xnat0/1 (strict need order); gpsimd = consts + wh, then
    steady xt prefetch self-throttled by the xt pool ring WAR. This
    plus the out-DMA dn-split (sync/gpsimd) keeps the early z stream
    fed; HAM un-throttles at ~18us and never re-cools.
  - pooling keeps the v2 4-strip tile_position layout but runs 2-way in
    practice (bf16 moving streams are SBUF-read-bandwidth capped at 2
    concurrent; fp8 streams get 4) -> ~14us, near the 13us floor of
    reading 16MB bf16 at 512B/cycle. Do NOT try fp8 x for pooling: the
    quantization noise exceeds the output tolerance.
  - 1/den folded via reciprocal + [1,512] tensor_scalar as in v2; the
    final out DMA writes per-dn halves directly from the scale result.
  - z DR stream unchanged: ~110us of fp8 DoubleRow matmuls = the PE
    hardware floor for this problem (DoubleRow is TRN2's only fp8 perf
    mode; MX is trn3-only). PE busy ~145us of ~164us span.

Failed experiments (do not retry blindly): PE warmup during the DMA
head (head is DMA-throughput-bound; a warm PE just starves and
re-cools, and undriven warmup MMs get dead-code-eliminated anyway);
xt1 quarters on the scalar queue interleaved with wy (starves wy
LDWs); one shared whhn psum tile (start=False pattern raced somewhere
-> b=0-per-core errors); DVE cross-quadrant tensor_add partial sums
(walrus verifier rejects partition-base-mismatched InstTensorTensor).
"""

import sys

if "/opt/trn_rl_repo" not in sys.path:
    sys.path.insert(0, "/opt/trn_rl_repo")

import numpy as np
import ml_dtypes

import concourse.bacc as bacc
import concourse.mybir as mybir
import concourse.tile as tile
from concourse.bass_utils import run_bass_kernel_spmd

F32 = mybir.dt.float32
BF16 = mybir.dt.bfloat16
FP8 = mybir.dt.float8e4
AF = mybir.ActivationFunctionType
DR = mybir.MatmulPerfMode.DoubleRow
MULT = mybir.AluOpType.mult
ADD = mybir.AluOpType.add

N_CORES = 8
B, T, D = 32, 2048, 1024
B_LOC = B // N_CORES          # 4 batches per core
NTOK = B_LOC * T              # 8192 tokens per core
TILE_T = 512                  # tokens per j-tile
NBT = NTOK // TILE_T          # 16 j-tiles
BT_PER_B = T // TILE_T        # 4 j-tiles per batch
CH = TILE_T // 128            # 4 128-token chunks per j-tile
KD = D // 128                 # 8 contraction chunks
EB = D // 128                 # 8 output-feature blocks
NH = NBT // 2                 # 8 halves (j-pairs)

_CACHE = {}


def build(cnt=(4, 4, 4, 4)):
    """cnt[b] = number of valid 512-token tiles for batch slot b.
    Fully-masked tiles beyond cnt[b] are skipped entirely (their
    attention weights are provably zero)."""
    nbt = sum(cnt)
    ntok = nbt * TILE_T
    # halves schedule: (slot, hh, is_last_half, first_tile, n_tiles).
    # Odd counts produce a trailing single-tile half (nj=1).
    assert all(1 <= c <= 4 for c in cnt) and cnt[0] >= 2
    sched = []
    o = 0
    for b, c in enumerate(cnt):
        nhh = (c + 1) // 2
        for hh in range(nhh):
            nj = 2 if 2 * hh + 2 <= c else 1
            sched.append((b, hh, hh == nhh - 1, o, nj))
            o += nj
    assert o == nbt

    nc = bacc.Bacc("TRN2", target_bir_lowering=False, debug=False,
                   num_devices=N_CORES)

    x = nc.dram_tensor("x", [ntok, D], BF16, kind="ExternalInput").ap()
    xt = nc.dram_tensor("xt", [nbt, 128, KD * TILE_T], FP8,
                        kind="ExternalInput").ap()
    wyt = nc.dram_tensor("wyt", [EB, 128, KD * 128], FP8,
                         kind="ExternalInput").ap()
    wht = nc.dram_tensor("wht", [EB, 128, KD * 128], FP8,
                         kind="ExternalInput").ap()
    CW = KD * B_LOC + EB * 32 + 1 + 1 + ntok // 128
    consts = nc.dram_tensor("consts", [128, CW], BF16,
                            kind="ExternalInput").ap()
    out = nc.dram_tensor("out", [B_LOC, D], F32, kind="ExternalOutput").ap()

    x4 = x.rearrange("(j c p) d -> j p c d", p=128, c=CH)

    with tile.TileContext(nc) as tc:
        with tc.tile_pool(name="consts", bufs=1) as cp, \
             tc.tile_pool(name="wy", bufs=1) as wyp, \
             tc.tile_pool(name="xnat", bufs=8) as xp, \
             tc.tile_pool(name="xt", bufs=4) as xtp, \
             tc.tile_pool(name="th", bufs=2) as thp, \
             tc.tile_pool(name="small", bufs=2) as smp, \
             tc.tile_pool(name="mps", bufs=1, space="PSUM") as psum:

            state = {}
            pending = []

            def pop1():
                if pending:
                    pending.pop(0)()

            def load_xt(j, split=1, eng=None):
                eng = eng or nc.gpsimd
                t = xtp.tile([128, KD * TILE_T], FP8, tag="xt",
                             name=f"xt_{j}")
                w = KD * TILE_T // split
                for s in range(split):
                    eng.dma_start(
                        t[:, s * w:(s + 1) * w],
                        xt[j][:, s * w:(s + 1) * w])
                state[("xt", j)] = t

            def load_xnat(j, eng=None):
                eng = eng or nc.sync
                t = xp.tile([128, CH * D], BF16, tag="xn", name=f"x_{j}")
                eng.dma_start(
                    t[:].rearrange("p (c d) -> p c d", c=CH), x4[j])
                state[("xn", j)] = t

            # ---- phase 0: DMAs spread across queues, first-needed-first.
            whp_cm = tc.tile_pool(name="wh", bufs=1)
            whp = whp_cm.__enter__()
            xt_t0 = xtp.tile([128, KD * TILE_T], FP8, tag="xt", name="xt_0")
            xt_t1 = xtp.tile([128, KD * TILE_T], FP8, tag="xt", name="xt_1")
            state[("xt", 0)] = xt_t0
            state[("xt", 1)] = xt_t1
            wy_sb = [wyp.tile([128, KD * 128], FP8, tag=f"wy{eb}",
                              name=f"wy_{eb}") for eb in range(EB)]
            wh_sb = [whp.tile([128, KD * 128], FP8, tag=f"wh{eb}",
                              name=f"wh_{eb}") for eb in range(EB)]
            consts_sb = cp.tile([128, CW], BF16)
            enc_sb = consts_sb[:, 0:KD * B_LOC]
            wa_sb = [consts_sb[:, KD * B_LOC + 32 * eb:
                               KD * B_LOC + 32 * (eb + 1)]
                     for eb in range(EB)]
            _o0 = KD * B_LOC + EB * 32
            ones_sb = consts_sb[:, _o0:_o0 + 1]
            hot_sb = consts_sb[:, _o0 + 1:_o0 + 2]
            mask_sb = consts_sb[:, _o0 + 2:_o0 + 2 + ntok // 128]
            half_sb = cp.tile([128, 1], F32)
            nc.vector.memset(half_sb[:], 0.5)
            whhn_sb = cp.tile([128, EB * B_LOC], F32)


            # sync queue: xt0/xt1 quarters (the z-stream critical path)
            QW = KD * TILE_T // 4
            for s in range(4):
                nc.sync.dma_start(xt_t0[:, s * QW:(s + 1) * QW],
                                  xt[0][:, s * QW:(s + 1) * QW])
                nc.sync.dma_start(xt_t1[:, s * QW:(s + 1) * QW],
                                  xt[1][:, s * QW:(s + 1) * QW])
            # scalar queue: wy weights (one needed every ~2.1us), then
            # the h1 xt tiles, then the first x-natural tiles -- strictly
            # in need order so the sync-queue xt quarters get bandwidth
            for s in range(EB):
                nc.scalar.dma_start(wy_sb[s][:], wyt[s])
            load_xt(2, eng=nc.scalar)
            load_xt(3, eng=nc.scalar)
            load_xnat(0, eng=nc.scalar)
            load_xnat(1, eng=nc.scalar)
            # gpsimd queue: ONE packed consts DMA + all wh (h0-critical);
            # steady-state xt prefetch follows and is self-throttled by
            # the xt pool ring (WAR on slot reuse)
            nc.gpsimd.dma_start(consts_sb[:], consts[:])
            for s in range(EB):
                nc.gpsimd.dma_start(wh_sb[s][:], wht[s])

            # ---- whhn: one eb at a time (interleaved into early z).
            # All eb groups write disjoint columns of ONE psum tile so
            # there is no WAR ring serialization between groups; the
            # psum->sbuf copy runs once per eb pair.
            def emit_whhn(eb):
                def fn():
                    php = psum.tile([128, B_LOC], F32, tag="small", bufs=1,
                                    name=f"php_{eb}")
                    for k in range(KD):
                        nc.tensor.matmul(
                            php[:], wh_sb[eb][:, k * 128:(k + 1) * 128],
                            enc_sb[:, k * B_LOC:(k + 1) * B_LOC],
                            start=(k == 0), stop=(k == KD - 1))
                    nc.vector.tensor_copy(
                        whhn_sb[:, eb * B_LOC:(eb + 1) * B_LOC], php[:])
                return fn

            # ---- z + tanh for one (half, eb): both j-tiles share LDW ----
            def get_th(h):
                key = ("th", h)
                if key not in state:
                    state[key] = thp.tile([128, EB * 2 * 512], FP8,
                                          tag="th", name=f"th_{h}")
                return state[key]

            def emit_z(h, eb, b, j0, nj, mid=None):
                zp = psum.tile([128, 2 * 512], F32, tag="z", bufs=2,
                               name=f"z_{h}_{eb}")
                for q in range(KD // 2):
                    w_ap = wy_sb[eb][:, q * 256:(q + 1) * 256].rearrange(
                        "p (i m) -> p i m", i=2)
                    for jl in range(nj):
                        xt_t = state[("xt", j0 + jl)]
                        mm = nc.tensor.matmul(
                            zp[:, jl * 512:(jl + 1) * 512], w_ap,
                            xt_t[:, 2 * q * TILE_T:(2 * q + 2) * TILE_T]
                            .rearrange("p (i n) -> p i n", i=2),
                            start=(q == 0), stop=(q == KD // 2 - 1),
                            perf_mode=DR)
                        if jl == 1:
                            mm.ldweights = False
                if mid is not None:
                    mid()
                # tanh over the half's j-tiles; bias whhn[:, eb, b]
                th_t = get_th(h)
                thv = th_t[:].rearrange("p (e jl n) -> p e jl n",
                                        e=EB, jl=2)
                zv = zp[:].rearrange("p (jl n) -> p jl n", jl=2)
                nc.scalar.activation(
                    thv[:, eb, 0:nj], zv[:, 0:nj],
                    AF.Tanh, scale=1.0 / 32.0,
                    bias=whhn_sb[:, eb * B_LOC + b:eb * B_LOC + b + 1])

            # ---- apre: 4-way col-tiled strips, one group per (h, eb).
            # Full-bank tile (exclusive has_written domain): the single
            # start=True on the first MM clears the bank; the other strips
            # first-touch-overwrite, then everything accumulates.
            def get_aps(h):
                key = ("aps", h)
                if key not in state:
                    t = psum.tile([128, 512], F32, tag="aps",
                                  bufs=1, name=f"aps_{h}")
                    nc.vector.memset(t[:, 0:256], 0.0)
                    state[key] = t
                return state[key]

            def emit_apre(h, eb):
                nj = state[("nj", h)]
                th_t = state[("th", h)]
                thv = (th_t[:].rearrange("p (e jl n) -> p e jl n",
                                         e=EB, jl=2)[:, :, 0:nj])
                aps = get_aps(h)
                apsv = (aps[:, 0:256].rearrange("p (jl n) -> p jl n", jl=2)
                        [:, 0:nj])
                for c in range(4):
                    nc.tensor.matmul(
                        apsv[32 * c:32 * (c + 1), :, :],
                        wa_sb[eb],
                        thv[:, eb, :, 128 * c:128 * (c + 1)],
                        start=False, stop=(eb == EB - 1),
                        tile_position=(0, 32 * c),
                        skip_group_check=True)
                if eb == EB - 1:
                    state.pop(("th", h))
                    state.pop(("nj", h))

            def emit_apre_batch(h):
                # all 8 eb groups back-to-back: drains pipeline, one
                # insertion point in the z stream instead of eight
                for eb in range(EB):
                    emit_apre(h, eb)

            # ---- per-half epilogue, interleaved between z groups ----
            def get_ew(b):
                key = ("ew", b)
                if key not in state:
                    state[key] = smp.tile([128, 4 * CH], BF16, tag="ew",
                                          name=f"ew_{b}")
                return state[key]

            def queue_h_epilogue(h, b, hh, last, bcnt, j0, nj):

                def t_trans():
                    aps = state.pop(("aps", h))
                    apct = smp.tile([128, 256], F32, tag="apct",
                                    name=f"apct_{h}")
                    nc.vector.transpose(apct[:], aps[:, 0:256])
                    state[("apct", h)] = apct

                def t_act():
                    apct = state.pop(("apct", h))
                    apcv = apct[:].rearrange("p (m e) -> p m e", m=8)
                    nm = 4 * nj
                    tj = smp.tile([128, 8], F32, tag="tj", name=f"tj_{h}")
                    nc.scalar.activation(tj[:, 0:nm], apcv[:, 0:nm, 0:1],
                                         AF.Tanh, scale=0.5)
                    ej = smp.tile([128, 8], F32, tag="ej", name=f"ej_{h}")
                    nc.scalar.activation(ej[:, 0:nm], tj[:, 0:nm], AF.Exp,
                                         bias=half_sb[:], scale=0.5)
                    ew = get_ew(b)
                    nc.vector.tensor_mul(
                        ew[:, hh * 8:hh * 8 + nm], ej[:, 0:nm],
                        mask_sb[:, 4 * j0:4 * j0 + nm])

                def t_den():
                    ew = state[("ew", b)]
                    acc = smp.tile([128, 1], F32, tag="acc",
                                   name=f"acc_{b}")
                    nc.vector.tensor_reduce(
                        acc[:], ew[:, 0:4 * bcnt],
                        mybir.AxisListType.XYZW, ADD)
                    accb = smp.tile([128, 1], BF16, tag="accb",
                                    name=f"accb_{b}")
                    nc.vector.tensor_copy(accb[:], acc[:])
                    den = psum.tile([1, 1], F32, tag="small", bufs=1,
                                    name=f"den_{b}")
                    nc.tensor.matmul(den[0:1, :], ones_sb,
                                     accb[:], start=True, stop=True)
                    rec = smp.tile([1, 1], F32, tag="rec", name=f"rec_{b}")
                    nc.vector.reciprocal(rec[:], den[0:1, :])
                    state[("rec", b)] = rec

                def t_pool(dn):
                    def fn():
                        ew = state[("ew", b)]
                        if hh == 0:
                            state[("num", b, dn)] = psum.tile(
                                [128, 512], F32, tag="num", bufs=2,
                                name=f"num_{b}_{dn}")
                            nc.vector.memset(state[("num", b, dn)][:], 0.0)
                        num = state[("num", b, dn)]
                        for jl in range(nj):
                            j = j0 + jl
                            xn = state[("xn", j)]
                            for k in range(CH):
                                col = (hh * 2 + jl) * CH + k
                                pos = k * 32
                                nc.tensor.matmul(
                                    num[pos:pos + 1, :],
                                    ew[:, col:col + 1],
                                    xn[:, k * D + dn * 512:
                                       k * D + (dn + 1) * 512],
                                    start=(col == 0),
                                    stop=(last and jl == nj - 1),
                                    tile_position=(0, pos),
                                    skip_group_check=True)
                            if dn == 1:
                                state.pop(("xn", j))
                    return fn

                def t_scale(dn):
                    def fn():
                        num = state.pop(("num", b, dn))
                        nsb = smp.tile([128, 512], BF16, tag="nsb",
                                       name=f"nsb_{b}_{dn}")
                        nc.vector.tensor_copy(nsb[:], num[:])
                        ns = psum.tile([1, 512], F32, tag="small", bufs=1,
                                       name=f"ns_{b}_{dn}")
                        nc.tensor.matmul(ns[0:1, :], hot_sb, nsb[:],
                                         start=True, stop=True)
                        rec = state[("rec", b)]
                        ob = smp.tile([1, 512], F32, tag="ob",
                                      name=f"ob_{b}_{dn}")
                        nc.vector.tensor_scalar_mul(ob[:], ns[0:1, :],
                                                    rec[:])
                        eng = nc.sync if dn == 0 else nc.gpsimd
                        eng.dma_start(
                            out[b:b + 1, dn * 512:(dn + 1) * 512], ob[:])
                        if dn == 1:
                            state.pop(("rec", b))
                            state.pop(("ew", b))
                    return fn

                pending.append(t_trans)
                pending.append(t_act)
                if last:
                    pending.append(t_den)
                pending.append(t_pool(0))
                pending.append(t_pool(1))
                if last:
                    pending.append(t_scale(0))
                    pending.append(t_scale(1))

            # ---- main loop over halves (slot schedule) ----
            nsched = len(sched)
            nxt = [4, 4]    # next xt / xnat tile to prefetch
            for h, (sb, hh, last, j0, nj) in enumerate(sched):
                state[("nj", h)] = nj
                if h == 1:
                    load_xnat(2, eng=nc.sync)
                    load_xnat(3, eng=nc.sync)
                for _ in range(2):
                    if nxt[0] < nbt:
                        load_xt(nxt[0])
                        nxt[0] += 1
                for _ in range(2):
                    if nxt[1] < nbt:
                        load_xnat(nxt[1])
                        nxt[1] += 1
                for eb in range(EB):
                    if h == 0:
                        emit_z(h, eb, sb, j0, nj, mid=emit_whhn(eb))
                    else:
                        emit_z(h, eb, sb, j0, nj)
                    if h == nsched - 1 and eb > 0:
                        # last half: apre trails per-eb so the tail is
                        # short once the z stream ends
                        emit_apre(h, eb - 1)
                    if h > 0 and eb == 0:
                        emit_apre_batch(h - 1)
                    if h > 0:
                        pop1()
                        if len(pending) > 5:
                            pop1()
                queue_h_epilogue(h, sb, hh, last, cnt[sb], j0, nj)
                if h == 0:
                    whp_cm.__exit__(None, None, None)
            emit_apre(nsched - 1, EB - 1)
            while pending:
                pop1()

    nc.compile()
    return nc


def _plan(mask):
    """Assign batches to (core, slot) by valid length. Slot shape
    (4,4,4,2) when at least N_CORES batches fit in 2 tiles, else the
    dense (4,4,4,4). Returns (cnt, assign) with assign[core][slot] =
    original batch index."""
    lengths = np.asarray(mask).sum(1).astype(int)
    tiles = np.maximum(1, -(-lengths // TILE_T))
    order = np.argsort(tiles, kind="stable")
    desc = order[::-1]
    td = tiles[desc]
    if (tiles <= 2).sum() >= N_CORES:
        # preferred: even halves only -- the single-tile-half variant
        # of (4,4,3,2) measured SLOWER than (4,4,4,2) despite one
        # fewer tile (pipeline fragmentation), so it is only a fallback
        pass
    if (tiles <= 2).sum() >= N_CORES:
        cnt = (4, 4, 4, 2)
        shorts = list(order[:N_CORES])
        longs = list(order[N_CORES:])
        assign = [[longs[3 * ci], longs[3 * ci + 1], longs[3 * ci + 2],
                   shorts[ci]] for ci in range(N_CORES)]
    elif (len(desc) == 32 and (td[16:24] <= 3).all()
            and (td[24:32] <= 2).all()):
        cnt = (4, 4, 3, 2)
        assign = [[desc[2 * ci], desc[2 * ci + 1], desc[16 + ci],
                   desc[24 + ci]] for ci in range(N_CORES)]
    else:
        cnt = (4, 4, 4, 4)
        assign = [[4 * ci, 4 * ci + 1, 4 * ci + 2, 4 * ci + 3]
                  for ci in range(N_CORES)]
    return cnt, assign


def _host_pack(full_input, encoding, mask, W_h, W_y, w_a, cnt, assign):
    """Per-core input maps (layout transforms / casts only)."""
    nbt = sum(cnt)
    ntok = nbt * TILE_T
    CW_ = KD * B_LOC + EB * 32 + 1 + 1 + ntok // 128
    wyT = np.ascontiguousarray(W_y.T)  # [d, e]
    whT = np.ascontiguousarray(W_h.T)
    wyt_rows = np.empty((EB, 128, KD * 128), ml_dtypes.float8_e4m3)
    wht_rows = np.empty((EB, 128, KD * 128), ml_dtypes.float8_e4m3)
    for eb in range(EB):
        for k in range(KD):
            wyt_rows[eb, :, k * 128:(k + 1) * 128] = (
                32.0 * wyT[k * 128:(k + 1) * 128, eb * 128:(eb + 1) * 128])
            wht_rows[eb, :, k * 128:(k + 1) * 128] = (
                32.0 * whT[k * 128:(k + 1) * 128, eb * 128:(eb + 1) * 128])
    in_maps = []
    for ci in range(N_CORES):
        gbs = assign[ci]
        # concatenate only the VALID tiles of each assigned batch
        xf = np.ascontiguousarray(np.concatenate(
            [full_input[gb, :cnt[b] * TILE_T]
             for b, gb in enumerate(gbs)], axis=0).astype(np.float32))
        mflat = np.concatenate(
            [mask[gb, :cnt[b] * TILE_T] for b, gb in enumerate(gbs)])
        # token permutation within each j-tile: stored (k, cc, i) holds
        # logical (cc, k, i) so the DVE block transpose of the apre strips
        # lands exactly on the pooling stationary layout.
        xperm = np.ascontiguousarray(
            xf.reshape(nbt, 4, 4, 32, D).transpose(0, 2, 1, 3, 4)
            .reshape(ntok, D))
        x_i = xperm.astype(ml_dtypes.bfloat16)
        xt_i = np.ascontiguousarray(
            xf.T.astype(ml_dtypes.float8_e4m3)      # [D, ntok], logical
            .reshape(KD, 128, nbt, TILE_T)
            .transpose(2, 1, 0, 3)                  # [j, p, k, t]
            .reshape(nbt, 128, KD * TILE_T))
        enc_sel = np.ascontiguousarray(encoding[gbs])    # slot order
        enc_i = ((1.0 / 32.0) * enc_sel.T.reshape(KD, 128, B_LOC)
                 .transpose(1, 0, 2).reshape(128, KD * B_LOC))
        mperm = (np.ascontiguousarray(mflat)
                 .reshape(nbt, 4, 4, 32).transpose(0, 2, 1, 3)
                 .reshape(ntok))
        mask_i = mperm.reshape(ntok // 128, 128).T
        consts_i = np.zeros((128, CW_), np.float32)
        consts_i[:, 0:KD * B_LOC] = enc_i
        for eb in range(EB):
            consts_i[:, KD * B_LOC + 32 * eb] = w_a[eb * 128:(eb + 1) * 128]
        o0 = KD * B_LOC + EB * 32
        consts_i[:, o0] = 1.0
        consts_i[[0, 32, 64, 96], o0 + 1] = 1.0
        consts_i[:, o0 + 2:o0 + 2 + ntok // 128] = mask_i
        in_maps.append({
            "x": x_i, "xt": xt_i, "wyt": wyt_rows, "wht": wht_rows,
            "consts": consts_i.astype(ml_dtypes.bfloat16),
        })
    return in_maps


def run(inputs, trace=False):
    cnt, assign = _plan(inputs["mask"])
    if cnt not in _CACHE:
        _CACHE[cnt] = build(cnt)
    nc = _CACHE[cnt]
    in_maps = _host_pack(**inputs, cnt=cnt, assign=assign)
    res = run_bass_kernel_spmd(nc, in_maps, core_ids=list(range(N_CORES)),
                               trace=trace)
    out = np.empty((B, D), np.float32)
    for ci in range(N_CORES):
        for b, gb in enumerate(assign[ci]):
            out[gb] = res.results[ci]["out"][b]
    return out, res


def kernel(**inputs):
    inputs = {k: np.asarray(v) for k, v in inputs.items()}
    out, _ = run(inputs, trace=False)
    return out
